# revision 8
# baseline (speedup 1.0000x reference)
"""Mesh2Grid GNN message passing kernel for 8 Trainium2 NeuronCores.

Strategy (data-parallel over edges, grid rows sharded by range):
  - Core k owns grid rows [k*12500, (k+1)*12500). Edges are bucketed to the
    core owning their dst; the scatter-sum is then fully core-local (no
    collectives).
  - Linear layers are commuted through the gather/scatter:
      A = mesh_x @ W1e_top + b1_e          (per-mesh-node, tiny)
      B = grid_slice @ W1e_bot             (per-grid-node, local slice)
      hidden[e] = relu(A[src[e]] + B[dst[e]])
      agg_hT = segment_sum_T(hidden)       (one-hot matmul per 512-row block)
      aggT = W2e^T @ agg_hT (+ deg x b2_e folded downstream)
      grid_new^T = W2g^T @ relu(W1gtop^T @ gxT + W1gbot^T @ aggT + bias)
  - Host: bucket/sort edges, build int16 gather indices, add the f32
    residual grid_x + grid_new at the end.
All device matmuls in bf16 with f32 PSUM accumulation.
"""

import math
import os
from contextlib import ExitStack

import numpy as np
import ml_dtypes

BF16 = ml_dtypes.bfloat16

# Problem constants (hardcoded per contract; kernel.py must be self-contained).
N_MESH = 10000
N_GRID = 100000
N_EDGE = 300000
D = 256
H = 256
NCORES = 8
GPC = N_GRID // NCORES          # grid rows per core
SB = 512                        # superblock width (dst rows per scatter group)
NSB = math.ceil(GPC / SB)       # superblocks per core
GPC_PAD = NSB * SB
NMESH_PAD = math.ceil(N_MESH / 128) * 128


def _ceil(a, b):
    return (a + b - 1) * b // b if False else math.ceil(a / b) * b


class _Cfg:
    """Geometry of one build (all cores share one NEFF)."""

    def __init__(self, tps, has_b1e, has_cbias, has_b2g,
                 nmesh_pad=NMESH_PAD, gpc_pad=GPC_PAD, nsb=NSB, sb=SB):
        self.tps = tps                  # 128-edge tiles per superblock
        self.eps = tps * 128            # edge slots per superblock
        self.has_b1e = has_b1e
        self.has_cbias = has_cbias
        self.has_b2g = has_b2g
        self.nmesh_pad = nmesh_pad
        self.gpc_pad = gpc_pad
        self.nsb = nsb
        self.sb = sb

    def key(self):
        return (self.tps, self.has_b1e, self.has_cbias, self.has_b2g,
                self.nmesh_pad, self.gpc_pad, self.nsb, self.sb)


_PROGRAM_CACHE = {}


def _build_program(cfg):
    import concourse.bass as bass
    import concourse.bacc as bacc
    import concourse.mybir as mybir
    import concourse.tile as tile

    dt = mybir.dt
    Alu = mybir.AluOpType
    Act = mybir.ActivationFunctionType

    NM, GP, nsb, sb, TPS, EPS = (cfg.nmesh_pad, cfg.gpc_pad, cfg.nsb, cfg.sb,
                                 cfg.tps, cfg.eps)
    NE = nsb * EPS                 # total edge slots per core

    nc = bacc.Bacc("TRN2", target_bir_lowering=False, debug=False,
                   enable_asserts=False, num_devices=NCORES)

    # ---- I/O ----
    mesh_xT = nc.dram_tensor("mesh_xT", [D, NM], dt.bfloat16, kind="ExternalInput")
    gxT_d = nc.dram_tensor("gxT", [D, GP], dt.bfloat16, kind="ExternalInput")
    w_names = ["w1e_top", "w1e_bot", "w2e", "w1g_top", "w1g_bot", "w2g"]
    w_d = {n: nc.dram_tensor(n, [D, H], dt.bfloat16, kind="ExternalInput")
           for n in w_names}
    idxA_d = nc.dram_tensor("idxA", [128, NE // 16], dt.int16, kind="ExternalInput")
    idxB_d = nc.dram_tensor("idxB", [128, NE // 16], dt.int16, kind="ExternalInput")
    dstf_d = nc.dram_tensor("dstf", [128, NE // 128], dt.float32, kind="ExternalInput")
    iota_d = nc.dram_tensor("iota", [128, sb], dt.float16, kind="ExternalInput")
    if cfg.has_b1e:
        b1e_d = nc.dram_tensor("b1e", [1, H], dt.bfloat16, kind="ExternalInput")
    if cfg.has_cbias:
        cbias_d = nc.dram_tensor("cbias", [2, H], dt.bfloat16, kind="ExternalInput")
    if cfg.has_b2g:
        b2g_d = nc.dram_tensor("b2g", [2, H], dt.bfloat16, kind="ExternalInput")
    if cfg.has_cbias or cfg.has_b2g:
        brhs_d = nc.dram_tensor("brhs", [2, GP], dt.bfloat16, kind="ExternalInput")

    outT_d = nc.dram_tensor("outT", [D, GP], dt.float32, kind="ExternalOutput")

    A_d = nc.dram_tensor("A_scr", [NM, H], dt.bfloat16, kind="Internal")
    B_d = nc.dram_tensor("B_scr", [GP, H], dt.bfloat16, kind="Internal")

    def cpn(ap):  # [ (c p) n ] dram -> [p c n] view for 128-partition loads
        return ap.rearrange("(c p) n -> p c n", c=2)

    with tile.TileContext(nc) as tc, ExitStack() as ctx:
        const = ctx.enter_context(tc.tile_pool(name="const", bufs=1))

        # Resident constants.
        gxT = const.tile([128, 2, GP], dt.bfloat16)
        nc.sync.dma_start(gxT[:], cpn(gxT_d.ap()))
        w = {}
        for n in w_names:
            w[n] = const.tile([128, 2, H], dt.bfloat16, tag=f"w_{n}",
                              name=f"w_{n}")
            nc.sync.dma_start(w[n][:], cpn(w_d[n].ap()))
        idxA = const.tile([128, NE // 16], dt.int16, tag="idxA")
        nc.sync.dma_start(idxA[:], idxA_d.ap())
        idxB = const.tile([128, NE // 16], dt.int16, tag="idxB")
        nc.sync.dma_start(idxB[:], idxB_d.ap())
        dstf = const.tile([128, NE // 128], dt.float32, tag="dstf")
        nc.sync.dma_start(dstf[:], dstf_d.ap())
        iota = const.tile([128, sb], dt.float16, tag="iota")
        nc.sync.dma_start(iota[:], iota_d.ap())
        if cfg.has_b1e:
            ones = const.tile([1, 128], dt.bfloat16, tag="ones")
            nc.vector.memset(ones[:], 1.0)
            b1e = const.tile([1, H], dt.bfloat16, tag="b1e")
            nc.sync.dma_start(b1e[:], b1e_d.ap())
        if cfg.has_cbias:
            cbias = const.tile([2, H], dt.bfloat16, tag="cbias")
            nc.sync.dma_start(cbias[:], cbias_d.ap())
        if cfg.has_b2g:
            b2g = const.tile([2, H], dt.bfloat16, tag="b2g")
            nc.sync.dma_start(b2g[:], b2g_d.ap())
        if cfg.has_cbias or cfg.has_b2g:
            brhs = const.tile([2, GP], dt.bfloat16, tag="brhs")
            nc.sync.dma_start(brhs[:], brhs_d.ap())

        # ---- Phase A: A = mesh_x @ W1e_top (+b1e); B = grid_slice @ W1e_bot
        with tc.tile_pool(name="pha", bufs=3) as pa, \
             tc.tile_pool(name="pha_big", bufs=1) as pab, \
             tc.tile_pool(name="psa", bufs=4, space="PSUM") as psa:
            mesh_sb = pab.tile([128, 2, NM], dt.bfloat16)
            nc.sync.dma_start(mesh_sb[:], cpn(mesh_xT.ap()))

            def proj(src_sb, n_tiles, wname, dst_dram, with_b1e, grp):
                # out rows [t*128:(t+1)*128] = src[:, t-tile].T @ W (+ bias)
                for t0 in range(0, n_tiles, grp):
                    g = min(grp, n_tiles - t0)
                    osb = pa.tile([128, grp, H], dt.bfloat16, tag="projo")
                    for j in range(g):
                        t = t0 + j
                        ps = psa.tile([128, H], dt.float32, tag="projp")
                        for c in range(2):
                            nc.tensor.matmul(
                                ps[:], src_sb[:, c, t * 128:(t + 1) * 128],
                                w[wname][:, c, :],
                                start=(c == 0),
                                stop=(c == 1 and not with_b1e))
                        if with_b1e:
                            nc.tensor.matmul(ps[:], ones[:], b1e[:],
                                             start=False, stop=True)
                        nc.scalar.copy(osb[:, j, :], ps[:])
                    nc.sync.dma_start(
                        dst_dram.ap().rearrange("(t p) n -> p t n", p=128)
                        [:, t0:t0 + g, :],
                        osb[:, :g, :])

            proj(mesh_sb, NM // 128, "w1e_top", A_d, cfg.has_b1e, 4)
            proj(gxT, GP // 128, "w1e_bot", B_d, False, 4)

        # ---- Main loop: per superblock gather -> hidden -> scatter -> MLP
        with tc.tile_pool(name="pg", bufs=2) as pg, \
             tc.tile_pool(name="ph", bufs=2) as ph, \
             tc.tile_pool(name="pS", bufs=3) as pS, \
             tc.tile_pool(name="pT", bufs=2) as pT, \
             tc.tile_pool(name="po", bufs=2) as po, \
             tc.tile_pool(name="ps_ah", bufs=1, space="PSUM") as ps_ah, \
             tc.tile_pool(name="ps_at", bufs=1, space="PSUM") as ps_at, \
             tc.tile_pool(name="ps_ct", bufs=1, space="PSUM") as ps_ct, \
             tc.tile_pool(name="ps_o", bufs=1, space="PSUM") as ps_o:

            for s in range(nsb):
                i16a, i16b = s * EPS // 16, (s + 1) * EPS // 16
                gA = pg.tile([128, TPS, H], dt.bfloat16, tag="gA")
                nc.gpsimd.dma_gather(gA[:], A_d.ap(), idxA[:, i16a:i16b],
                                     EPS, EPS, H, single_packet=False)
                gB = pg.tile([128, TPS, H], dt.bfloat16, tag="gB")
                nc.gpsimd.dma_gather(gB[:], B_d.ap(), idxB[:, i16a:i16b],
                                     EPS, EPS, H, single_packet=False)
                tmp = ph.tile([128, TPS, H], dt.bfloat16, tag="tmp")
                nc.vector.tensor_tensor(tmp[:], gA[:], gB[:], Alu.add)
                hid = ph.tile([128, TPS, H], dt.bfloat16, tag="hid")
                nc.scalar.activation(hid[:], tmp[:], Act.Relu)

                # scatter: agg_hT[f, d] += sum_e hid[e, f] * onehot[e, d]
                pah = ps_ah.tile([128, 2, sb], dt.float32, tag="pah")
                for t in range(TPS):
                    S = pS.tile([128, sb], dt.bfloat16, tag="S")
                    nc.vector.tensor_scalar(
                        S[:], iota[:],
                        dstf[:, s * TPS + t: s * TPS + t + 1],
                        -float(s * sb), Alu.subtract, Alu.is_equal)
                    for c in range(2):
                        nc.tensor.matmul(
                            pah[:, c, :], hid[:, t, c * 128:(c + 1) * 128],
                            S[:], start=(t == 0), stop=(t == TPS - 1))
                ahT = pT.tile([128, 2, sb], dt.bfloat16, tag="ahT")
                nc.scalar.copy(ahT[:], pah[:])

                # aggT[h, d] = sum_f w2e[f, h] * ahT[f, d]
                pat = ps_at.tile([128, 2, sb], dt.float32, tag="pat")
                for hc in range(2):
                    for fc in range(2):
                        nc.tensor.matmul(
                            pat[:, hc, :],
                            w["w2e"][:, fc, hc * 128:(hc + 1) * 128],
                            ahT[:, fc, :], start=(fc == 0), stop=(fc == 1))
                aT = pT.tile([128, 2, sb], dt.bfloat16, tag="aT")
                nc.scalar.copy(aT[:], pat[:])

                # CT[h, d] = W1g_top^T gxT + W1g_bot^T aggT (+ bias terms)
                pct = ps_ct.tile([128, 2, sb], dt.float32, tag="pct")
                dlo, dhi = s * sb, (s + 1) * sb
                for hc in range(2):
                    hsl = slice(hc * 128, (hc + 1) * 128)
                    for c in range(2):
                        nc.tensor.matmul(pct[:, hc, :], w["w1g_top"][:, c, hsl],
                                         gxT[:, c, dlo:dhi],
                                         start=(c == 0), stop=False)
                    for c in range(2):
                        nc.tensor.matmul(pct[:, hc, :], w["w1g_bot"][:, c, hsl],
                                         aT[:, c, :], start=False,
                                         stop=(c == 1 and not cfg.has_cbias))
                    if cfg.has_cbias:
                        nc.tensor.matmul(pct[:, hc, :], cbias[:, hsl],
                                         brhs[:, dlo:dhi],
                                         start=False, stop=True)
                h1T = pT.tile([128, 2, sb], dt.bfloat16, tag="h1T")
                nc.scalar.activation(h1T[:], pct[:], Act.Relu)

                # outT[h, d] = sum_f2 w2g[f2, h] * h1T[f2, d] (+ b2g)
                pso = ps_o.tile([128, 2, sb], dt.float32, tag="pso")
                for hc in range(2):
                    hsl = slice(hc * 128, (hc + 1) * 128)
                    for c in range(2):
                        nc.tensor.matmul(pso[:, hc, :], w["w2g"][:, c, hsl],
                                         h1T[:, c, :], start=(c == 0),
                                         stop=(c == 1 and not cfg.has_b2g))
                    if cfg.has_b2g:
                        # row 0 of b2g is zeros; pairs with the deg row of brhs
                        nc.tensor.matmul(pso[:, hc, :], b2g[:, hsl],
                                         brhs[:, dlo:dhi],
                                         start=False, stop=True)
                osb = po.tile([128, 2, sb], dt.float32, tag="osb")
                nc.vector.tensor_copy(osb[:], pso[:])
                for c in range(2):
                    nc.sync.dma_start(outT_d[c * 128:(c + 1) * 128, dlo:dhi],
                                      osb[:, c, :])

    nc.compile()
    return nc


def _prep_inputs(mesh_x, grid_x, edge_src, edge_dst,
                 w1_e, b1_e, w2_e, b2_e, w1_g, b1_g, w2_g, b2_g):
    """Host-side sharding/bucketing. Returns (cfg, in_maps)."""
    f32 = np.float32
    mesh_x = np.asarray(mesh_x, f32)
    grid_x = np.asarray(grid_x, f32)
    edge_src = np.asarray(edge_src, np.int32)
    edge_dst = np.asarray(edge_dst, np.int32)
    w1_e = np.asarray(w1_e, f32); b1_e = np.asarray(b1_e, f32)
    w2_e = np.asarray(w2_e, f32); b2_e = np.asarray(b2_e, f32)
    w1_g = np.asarray(w1_g, f32); b1_g = np.asarray(b1_g, f32)
    w2_g = np.asarray(w2_g, f32); b2_g = np.asarray(b2_g, f32)

    has_b1e = bool(np.any(b1_e != 0))
    has_b2e = bool(np.any(b2_e != 0))
    has_b1g = bool(np.any(b1_g != 0))
    has_b2g = bool(np.any(b2_g != 0))
    has_cbias = has_b2e or has_b1g

    core_of = edge_dst // GPC
    per_core = []
    max_cnt = 1
    for k in range(NCORES):
        sel = np.nonzero(core_of == k)[0]
        d = edge_dst[sel] - k * GPC
        order = np.argsort(d, kind="stable")
        s_srt = edge_src[sel][order].astype(np.int64)
        d_srt = d[order].astype(np.int64)
        sbid = d_srt // SB
        counts = np.bincount(sbid, minlength=NSB)
        max_cnt = max(max_cnt, int(counts.max(initial=0)))
        per_core.append((s_srt, d_srt, sbid, counts))

    tps = math.ceil(max_cnt / 128)
    cfg = _Cfg(tps, has_b1e, has_cbias, has_b2g,
               nmesh_pad=NMESH_PAD, gpc_pad=GPC_PAD, nsb=NSB, sb=SB)
    EPS = cfg.eps
    NE = NSB * EPS

    mesh_xT = np.zeros((D, NMESH_PAD), BF16)
    mesh_xT[:, :N_MESH] = mesh_x.T
    iota = np.broadcast_to(np.arange(SB, dtype=np.float16), (128, SB)).copy()

    shared = {
        "mesh_xT": mesh_xT,
        "w1e_top": w1_e[:D].astype(BF16),
        "w1e_bot": w1_e[D:].astype(BF16),
        "w2e": w2_e.astype(BF16),
        "w1g_top": w1_g[:D].astype(BF16),
        "w1g_bot": w1_g[D:].astype(BF16),
        "w2g": w2_g.astype(BF16),
        "iota": iota,
    }
    if has_b1e:
        shared["b1e"] = b1_e.reshape(1, H).astype(BF16)
    if has_cbias:
        shared["cbias"] = np.stack([b2_e @ w1_g[D:], b1_g]).astype(BF16)
    if has_b2g:
        shared["b2g"] = np.stack([np.zeros(H, np.float32), b2_g]).astype(BF16)

    in_maps = []
    for k in range(NCORES):
        s_srt, d_srt, sbid, counts = per_core[k]
        n = len(d_srt)
        grp_start = np.searchsorted(sbid, np.arange(NSB))
        rank = np.arange(n) - grp_start[sbid]
        pos = sbid * EPS + rank
        srcP = np.zeros(NE, np.int16)
        dstI = np.zeros(NE, np.int16)
        dstF = np.full(NE, -1e9, np.float32)
        srcP[pos] = s_srt.astype(np.int16)
        dstI[pos] = d_srt.astype(np.int16)
        dstF[pos] = d_srt.astype(np.float32)

        idxA = np.tile(srcP.reshape(-1, 16).T, (8, 1)).copy()
        idxB = np.tile(dstI.reshape(-1, 16).T, (8, 1)).copy()
        dstf = dstF.reshape(-1, 128).T.copy()

        gxT = np.zeros((D, GPC_PAD), BF16)
        gxT[:, :GPC] = grid_x[k * GPC:(k + 1) * GPC].T

        m = dict(shared)
        m.update(gxT=gxT, idxA=idxA, idxB=idxB, dstf=dstf)
        if has_cbias or has_b2g:
            deg = np.bincount(d_srt, minlength=GPC_PAD).astype(np.float32)
            m["brhs"] = np.stack([deg, np.ones(GPC_PAD, np.float32)]).astype(BF16)
        in_maps.append(m)
    return cfg, in_maps


def _run(inputs, trace=False, trace_kwargs=None):
    from concourse import bass_utils

    cfg, in_maps = _prep_inputs(**inputs)
    key = cfg.key()
    if key not in _PROGRAM_CACHE:
        _PROGRAM_CACHE[key] = _build_program(cfg)
    nc = _PROGRAM_CACHE[key]

    res = bass_utils.run_bass_kernel_spmd(
        nc, in_maps, core_ids=list(range(NCORES)), trace=trace,
        **(trace_kwargs or {}))

    grid_x = np.asarray(inputs["grid_x"], np.float32)
    out = np.empty((N_GRID, D), np.float32)
    for k in range(NCORES):
        outT = np.asarray(res.results[k]["outT"], np.float32)
        out[k * GPC:(k + 1) * GPC] = outT[:, :GPC].T
    out += grid_x
    return out, res


def kernel(**inputs) -> np.ndarray:
    out, _ = _run(inputs, trace=False)
    return out


# revision 10
# speedup vs baseline: 89.7130x; 89.7130x over previous
"""Mesh2Grid GNN message passing kernel for 8 Trainium2 NeuronCores.

Strategy (data-parallel over edges, grid rows sharded by range):
  - Core k owns grid rows [k*12500, (k+1)*12500). Edges are bucketed to the
    core owning their dst; the scatter-sum is then fully core-local (no
    collectives).
  - Linear layers are commuted through the gather/scatter:
      A = mesh_x @ W1e_top + b1_e          (per-mesh-node, tiny)
      B = grid_slice @ W1e_bot             (per-grid-node, local slice)
      hidden[e] = relu(A[src[e]] + B[dst[e]])
      agg_hT = segment_sum_T(hidden)       (one-hot matmul per 512-row block)
      aggT = W2e^T @ agg_hT (+ deg x b2_e folded downstream)
      grid_new^T = W2g^T @ relu(W1gtop^T @ gxT + W1gbot^T @ aggT + bias)
  - Host: bucket/sort edges, build int16 gather indices, add the f32
    residual grid_x + grid_new at the end.
All device matmuls in bf16 with f32 PSUM accumulation.
"""

import math
import os
from contextlib import ExitStack

import numpy as np
import ml_dtypes

BF16 = ml_dtypes.bfloat16

# Problem constants (hardcoded per contract; kernel.py must be self-contained).
N_MESH = 10000
N_GRID = 100000
N_EDGE = 300000
D = 256
H = 256
NCORES = 8
GPC = N_GRID // NCORES          # grid rows per core
SB = 512                        # superblock width (dst rows per scatter group)
NSB = math.ceil(GPC / SB)       # superblocks per core
GPC_PAD = NSB * SB
NMESH_PAD = math.ceil(N_MESH / 128) * 128


def _ceil(a, b):
    return (a + b - 1) * b // b if False else math.ceil(a / b) * b


class _Cfg:
    """Geometry of one build (all cores share one NEFF)."""

    def __init__(self, tps, has_b1e, has_cbias, has_b2g,
                 nmesh_pad=NMESH_PAD, gpc_pad=GPC_PAD, nsb=NSB, sb=SB):
        self.tps = tps                  # 128-edge tiles per superblock
        self.eps = tps * 128            # edge slots per superblock
        self.has_b1e = has_b1e
        self.has_cbias = has_cbias
        self.has_b2g = has_b2g
        self.nmesh_pad = nmesh_pad
        self.gpc_pad = gpc_pad
        self.nsb = nsb
        self.sb = sb

    def key(self):
        return (self.tps, self.has_b1e, self.has_cbias, self.has_b2g,
                self.nmesh_pad, self.gpc_pad, self.nsb, self.sb)


_PROGRAM_CACHE = {}


def _build_program(cfg, reps=1):
    """reps>1 repeats the whole compute body inside one NEFF (timing only)."""
    import concourse.bass as bass
    import concourse.bacc as bacc
    import concourse.mybir as mybir
    import concourse.tile as tile

    dt = mybir.dt
    Alu = mybir.AluOpType
    Act = mybir.ActivationFunctionType

    NM, GP, nsb, sb, TPS, EPS = (cfg.nmesh_pad, cfg.gpc_pad, cfg.nsb, cfg.sb,
                                 cfg.tps, cfg.eps)
    NE = nsb * EPS                 # total edge slots per core

    nc = bacc.Bacc("TRN2", target_bir_lowering=False, debug=False,
                   enable_asserts=False, num_devices=NCORES)

    # ---- I/O ----
    mesh_xT = nc.dram_tensor("mesh_xT", [D, NM], dt.bfloat16, kind="ExternalInput")
    gxT_d = nc.dram_tensor("gxT", [D, GP], dt.bfloat16, kind="ExternalInput")
    w_names = ["w1e_top", "w1e_bot", "w2e", "w1g_top", "w1g_bot", "w2g"]
    w_d = {n: nc.dram_tensor(n, [D, H], dt.bfloat16, kind="ExternalInput")
           for n in w_names}
    idxA_d = nc.dram_tensor("idxA", [128, NE // 16], dt.int16, kind="ExternalInput")
    idxB_d = nc.dram_tensor("idxB", [128, NE // 16], dt.int16, kind="ExternalInput")
    dstf_d = nc.dram_tensor("dstf", [128, NE // 128], dt.float32, kind="ExternalInput")
    iota_d = nc.dram_tensor("iota", [128, sb], dt.float16, kind="ExternalInput")
    if cfg.has_b1e:
        b1e_d = nc.dram_tensor("b1e", [1, H], dt.bfloat16, kind="ExternalInput")
    if cfg.has_cbias:
        cbias_d = nc.dram_tensor("cbias", [2, H], dt.bfloat16, kind="ExternalInput")
    if cfg.has_b2g:
        b2g_d = nc.dram_tensor("b2g", [2, H], dt.bfloat16, kind="ExternalInput")
    if cfg.has_cbias or cfg.has_b2g:
        brhs_d = nc.dram_tensor("brhs", [2, GP], dt.bfloat16, kind="ExternalInput")

    outT_d = nc.dram_tensor("outT", [D, GP], dt.float32, kind="ExternalOutput")

    A_d = nc.dram_tensor("A_scr", [NM, H], dt.bfloat16, kind="Internal")
    B_d = nc.dram_tensor("B_scr", [GP, H], dt.bfloat16, kind="Internal")

    def cpn(ap):  # [ (c p) n ] dram -> [p c n] view for 128-partition loads
        return ap.rearrange("(c p) n -> p c n", c=2)

    with tile.TileContext(nc) as tc, ExitStack() as ctx:
        const = ctx.enter_context(tc.tile_pool(name="const", bufs=1))

        # Resident constants.
        gxT = const.tile([128, 2, GP], dt.bfloat16)
        nc.sync.dma_start(gxT[:], cpn(gxT_d.ap()))
        w = {}
        for n in w_names:
            w[n] = const.tile([128, 2, H], dt.bfloat16, tag=f"w_{n}",
                              name=f"w_{n}")
            nc.sync.dma_start(w[n][:], cpn(w_d[n].ap()))
        idxA = const.tile([128, NE // 16], dt.int16, tag="idxA")
        nc.sync.dma_start(idxA[:], idxA_d.ap())
        idxB = const.tile([128, NE // 16], dt.int16, tag="idxB")
        nc.sync.dma_start(idxB[:], idxB_d.ap())
        dstf = const.tile([128, NE // 128], dt.float32, tag="dstf")
        nc.sync.dma_start(dstf[:], dstf_d.ap())
        iota = const.tile([128, sb], dt.float16, tag="iota")
        nc.sync.dma_start(iota[:], iota_d.ap())
        if cfg.has_b1e:
            ones = const.tile([1, 128], dt.bfloat16, tag="ones")
            nc.vector.memset(ones[:], 1.0)
            b1e = const.tile([1, H], dt.bfloat16, tag="b1e")
            nc.sync.dma_start(b1e[:], b1e_d.ap())
        if cfg.has_cbias:
            cbias = const.tile([2, H], dt.bfloat16, tag="cbias")
            nc.sync.dma_start(cbias[:], cbias_d.ap())
        if cfg.has_b2g:
            b2g = const.tile([2, H], dt.bfloat16, tag="b2g")
            nc.sync.dma_start(b2g[:], b2g_d.ap())
        if cfg.has_cbias or cfg.has_b2g:
            brhs = const.tile([2, GP], dt.bfloat16, tag="brhs")
            nc.sync.dma_start(brhs[:], brhs_d.ap())

        # ---- Phase A: A = mesh_x @ W1e_top (+b1e); B = grid_slice @ W1e_bot
        for _rep in range(reps):
            _phases(nc, tc, cfg, w, gxT, idxA, idxB, dstf, iota,
                    locals().get("ones"), locals().get("b1e"),
                    locals().get("cbias"), locals().get("b2g"),
                    locals().get("brhs"),
                    mesh_xT, A_d, B_d, outT_d, cpn)

    nc.compile()
    return nc


def _phases(nc, tc, cfg, w, gxT, idxA, idxB, dstf, iota, ones, b1e, cbias,
            b2g, brhs, mesh_xT, A_d, B_d, outT_d, cpn):
    import concourse.mybir as mybir
    dt = mybir.dt
    Alu = mybir.AluOpType
    Act = mybir.ActivationFunctionType
    NM, GP, nsb, sb, TPS, EPS = (cfg.nmesh_pad, cfg.gpc_pad, cfg.nsb, cfg.sb,
                                 cfg.tps, cfg.eps)
    if True:
        with tc.tile_pool(name="pha", bufs=3) as pa, \
             tc.tile_pool(name="pha_big", bufs=1) as pab, \
             tc.tile_pool(name="psa", bufs=4, space="PSUM") as psa:
            mesh_sb = pab.tile([128, 2, NM], dt.bfloat16)
            nc.sync.dma_start(mesh_sb[:], cpn(mesh_xT.ap()))

            def proj(src_sb, n_tiles, wname, dst_dram, with_b1e, grp):
                # out rows [t*128:(t+1)*128] = src[:, t-tile].T @ W (+ bias)
                for t0 in range(0, n_tiles, grp):
                    g = min(grp, n_tiles - t0)
                    osb = pa.tile([128, grp, H], dt.bfloat16, tag="projo")
                    for j in range(g):
                        t = t0 + j
                        ps = psa.tile([128, H], dt.float32, tag="projp")
                        for c in range(2):
                            nc.tensor.matmul(
                                ps[:], src_sb[:, c, t * 128:(t + 1) * 128],
                                w[wname][:, c, :],
                                start=(c == 0),
                                stop=(c == 1 and not with_b1e))
                        if with_b1e:
                            nc.tensor.matmul(ps[:], ones[:], b1e[:],
                                             start=False, stop=True)
                        nc.scalar.copy(osb[:, j, :], ps[:])
                    nc.sync.dma_start(
                        dst_dram.ap().rearrange("(t p) n -> p t n", p=128)
                        [:, t0:t0 + g, :],
                        osb[:, :g, :])

            proj(mesh_sb, NM // 128, "w1e_top", A_d, cfg.has_b1e, 4)
            proj(gxT, GP // 128, "w1e_bot", B_d, False, 4)

        # ---- Main loop: per superblock gather -> hidden -> scatter -> MLP
        with tc.tile_pool(name="pg", bufs=2) as pg, \
             tc.tile_pool(name="ph", bufs=2) as ph, \
             tc.tile_pool(name="pS", bufs=3) as pS, \
             tc.tile_pool(name="pT", bufs=2) as pT, \
             tc.tile_pool(name="po", bufs=2) as po, \
             tc.tile_pool(name="ps_ah", bufs=1, space="PSUM") as ps_ah, \
             tc.tile_pool(name="ps_at", bufs=1, space="PSUM") as ps_at, \
             tc.tile_pool(name="ps_ct", bufs=1, space="PSUM") as ps_ct, \
             tc.tile_pool(name="ps_o", bufs=1, space="PSUM") as ps_o:

            for s in range(nsb):
                i16a, i16b = s * EPS // 16, (s + 1) * EPS // 16
                gA = pg.tile([128, TPS, H], dt.bfloat16, tag="gA")
                nc.gpsimd.dma_gather(gA[:], A_d.ap(), idxA[:, i16a:i16b],
                                     EPS, EPS, H, single_packet=False)
                gB = pg.tile([128, TPS, H], dt.bfloat16, tag="gB")
                nc.gpsimd.dma_gather(gB[:], B_d.ap(), idxB[:, i16a:i16b],
                                     EPS, EPS, H, single_packet=False)
                tmp = ph.tile([128, TPS, H], dt.bfloat16, tag="tmp")
                nc.vector.tensor_tensor(tmp[:], gA[:], gB[:], Alu.add)
                hid = ph.tile([128, TPS, H], dt.bfloat16, tag="hid")
                nc.scalar.activation(hid[:], tmp[:], Act.Relu)

                # scatter: agg_hT[f, d] += sum_e hid[e, f] * onehot[e, d]
                pah = ps_ah.tile([128, 2, sb], dt.float32, tag="pah")
                for t in range(TPS):
                    S = pS.tile([128, sb], dt.bfloat16, tag="S")
                    nc.vector.tensor_scalar(
                        S[:], iota[:],
                        dstf[:, s * TPS + t: s * TPS + t + 1],
                        -float(s * sb), Alu.subtract, Alu.is_equal)
                    for c in range(2):
                        nc.tensor.matmul(
                            pah[:, c, :], hid[:, t, c * 128:(c + 1) * 128],
                            S[:], start=(t == 0), stop=(t == TPS - 1))
                ahT = pT.tile([128, 2, sb], dt.bfloat16, tag="ahT")
                nc.scalar.copy(ahT[:], pah[:])

                # aggT[h, d] = sum_f w2e[f, h] * ahT[f, d]
                pat = ps_at.tile([128, 2, sb], dt.float32, tag="pat")
                for hc in range(2):
                    for fc in range(2):
                        nc.tensor.matmul(
                            pat[:, hc, :],
                            w["w2e"][:, fc, hc * 128:(hc + 1) * 128],
                            ahT[:, fc, :], start=(fc == 0), stop=(fc == 1))
                aT = pT.tile([128, 2, sb], dt.bfloat16, tag="aT")
                nc.scalar.copy(aT[:], pat[:])

                # CT[h, d] = W1g_top^T gxT + W1g_bot^T aggT (+ bias terms)
                pct = ps_ct.tile([128, 2, sb], dt.float32, tag="pct")
                dlo, dhi = s * sb, (s + 1) * sb
                for hc in range(2):
                    hsl = slice(hc * 128, (hc + 1) * 128)
                    for c in range(2):
                        nc.tensor.matmul(pct[:, hc, :], w["w1g_top"][:, c, hsl],
                                         gxT[:, c, dlo:dhi],
                                         start=(c == 0), stop=False)
                    for c in range(2):
                        nc.tensor.matmul(pct[:, hc, :], w["w1g_bot"][:, c, hsl],
                                         aT[:, c, :], start=False,
                                         stop=(c == 1 and not cfg.has_cbias))
                    if cfg.has_cbias:
                        nc.tensor.matmul(pct[:, hc, :], cbias[:, hsl],
                                         brhs[:, dlo:dhi],
                                         start=False, stop=True)
                h1T = pT.tile([128, 2, sb], dt.bfloat16, tag="h1T")
                nc.scalar.activation(h1T[:], pct[:], Act.Relu)

                # outT[h, d] = sum_f2 w2g[f2, h] * h1T[f2, d] (+ b2g)
                pso = ps_o.tile([128, 2, sb], dt.float32, tag="pso")
                for hc in range(2):
                    hsl = slice(hc * 128, (hc + 1) * 128)
                    for c in range(2):
                        nc.tensor.matmul(pso[:, hc, :], w["w2g"][:, c, hsl],
                                         h1T[:, c, :], start=(c == 0),
                                         stop=(c == 1 and not cfg.has_b2g))
                    if cfg.has_b2g:
                        # row 0 of b2g is zeros; pairs with the deg row of brhs
                        nc.tensor.matmul(pso[:, hc, :], b2g[:, hsl],
                                         brhs[:, dlo:dhi],
                                         start=False, stop=True)
                osb = po.tile([128, 2, sb], dt.float32, tag="osb")
                nc.vector.tensor_copy(osb[:], pso[:])
                for c in range(2):
                    nc.sync.dma_start(outT_d[c * 128:(c + 1) * 128, dlo:dhi],
                                      osb[:, c, :])

    nc.compile()
    return nc


def _prep_inputs(mesh_x, grid_x, edge_src, edge_dst,
                 w1_e, b1_e, w2_e, b2_e, w1_g, b1_g, w2_g, b2_g):
    """Host-side sharding/bucketing. Returns (cfg, in_maps)."""
    f32 = np.float32
    mesh_x = np.asarray(mesh_x, f32)
    grid_x = np.asarray(grid_x, f32)
    edge_src = np.asarray(edge_src, np.int32)
    edge_dst = np.asarray(edge_dst, np.int32)
    w1_e = np.asarray(w1_e, f32); b1_e = np.asarray(b1_e, f32)
    w2_e = np.asarray(w2_e, f32); b2_e = np.asarray(b2_e, f32)
    w1_g = np.asarray(w1_g, f32); b1_g = np.asarray(b1_g, f32)
    w2_g = np.asarray(w2_g, f32); b2_g = np.asarray(b2_g, f32)

    has_b1e = bool(np.any(b1_e != 0))
    has_b2e = bool(np.any(b2_e != 0))
    has_b1g = bool(np.any(b1_g != 0))
    has_b2g = bool(np.any(b2_g != 0))
    has_cbias = has_b2e or has_b1g

    core_of = edge_dst // GPC
    per_core = []
    max_cnt = 1
    for k in range(NCORES):
        sel = np.nonzero(core_of == k)[0]
        d = edge_dst[sel] - k * GPC
        order = np.argsort(d, kind="stable")
        s_srt = edge_src[sel][order].astype(np.int64)
        d_srt = d[order].astype(np.int64)
        sbid = d_srt // SB
        counts = np.bincount(sbid, minlength=NSB)
        max_cnt = max(max_cnt, int(counts.max(initial=0)))
        per_core.append((s_srt, d_srt, sbid, counts))

    tps = math.ceil(max_cnt / 128)
    cfg = _Cfg(tps, has_b1e, has_cbias, has_b2g,
               nmesh_pad=NMESH_PAD, gpc_pad=GPC_PAD, nsb=NSB, sb=SB)
    EPS = cfg.eps
    NE = NSB * EPS

    mesh_xT = np.zeros((D, NMESH_PAD), BF16)
    mesh_xT[:, :N_MESH] = mesh_x.T
    iota = np.broadcast_to(np.arange(SB, dtype=np.float16), (128, SB)).copy()

    shared = {
        "mesh_xT": mesh_xT,
        "w1e_top": w1_e[:D].astype(BF16),
        "w1e_bot": w1_e[D:].astype(BF16),
        "w2e": w2_e.astype(BF16),
        "w1g_top": w1_g[:D].astype(BF16),
        "w1g_bot": w1_g[D:].astype(BF16),
        "w2g": w2_g.astype(BF16),
        "iota": iota,
    }
    if has_b1e:
        shared["b1e"] = b1_e.reshape(1, H).astype(BF16)
    if has_cbias:
        shared["cbias"] = np.stack([b2_e @ w1_g[D:], b1_g]).astype(BF16)
    if has_b2g:
        shared["b2g"] = np.stack([np.zeros(H, np.float32), b2_g]).astype(BF16)

    in_maps = []
    for k in range(NCORES):
        s_srt, d_srt, sbid, counts = per_core[k]
        n = len(d_srt)
        grp_start = np.searchsorted(sbid, np.arange(NSB))
        rank = np.arange(n) - grp_start[sbid]
        pos = sbid * EPS + rank
        srcP = np.zeros(NE, np.int16)
        dstI = np.zeros(NE, np.int16)
        dstF = np.full(NE, -1e9, np.float32)
        srcP[pos] = s_srt.astype(np.int16)
        dstI[pos] = d_srt.astype(np.int16)
        dstF[pos] = d_srt.astype(np.float32)

        idxA = np.tile(srcP.reshape(-1, 16).T, (8, 1)).copy()
        idxB = np.tile(dstI.reshape(-1, 16).T, (8, 1)).copy()
        dstf = dstF.reshape(-1, 128).T.copy()

        gxT = np.zeros((D, GPC_PAD), BF16)
        gxT[:, :GPC] = grid_x[k * GPC:(k + 1) * GPC].T

        m = dict(shared)
        m.update(gxT=gxT, idxA=idxA, idxB=idxB, dstf=dstf)
        if has_cbias or has_b2g:
            deg = np.bincount(d_srt, minlength=GPC_PAD).astype(np.float32)
            m["brhs"] = np.stack([deg, np.ones(GPC_PAD, np.float32)]).astype(BF16)
        in_maps.append(m)
    return cfg, in_maps


def _run(inputs, trace=False, trace_kwargs=None):
    from concourse import bass_utils

    cfg, in_maps = _prep_inputs(**inputs)
    key = cfg.key()
    if key not in _PROGRAM_CACHE:
        _PROGRAM_CACHE[key] = _build_program(cfg)
    nc = _PROGRAM_CACHE[key]

    res = bass_utils.run_bass_kernel_spmd(
        nc, in_maps, core_ids=list(range(NCORES)), trace=trace,
        **(trace_kwargs or {}))

    grid_x = np.asarray(inputs["grid_x"], np.float32)
    out = np.empty((N_GRID, D), np.float32)
    for k in range(NCORES):
        outT = np.asarray(res.results[k]["outT"], np.float32)
        out[k * GPC:(k + 1) * GPC] = outT[:, :GPC].T
    out += grid_x
    return out, res


def kernel(**inputs) -> np.ndarray:
    out, _ = _run(inputs, trace=False)
    return out
